# revision 1
# baseline (speedup 1.0000x reference)
"""GQA attention (B=4,S=2048,D=2048,H=16,KH=4) + RoPE + causal mask on 8 trn2 cores.

Sharding: 8 cores = 4 batches x 2 sequence-halves (causally balanced pairing:
parity0 owns q-chunks {0,3}, parity1 owns {1,2}; each core computes K/V for the
full sequence, attention + output projection only for its 1024 owned q rows).

Per-core pipeline (fp16 matmuls, fp32 accumulate/softmax):
  x (perm-rows, fp16) --DMA-transpose--> xT tiles
  K/V projections -> rope(K) -> kT [hd, s] fp16 resident; V [s, hd|1] fp16 resident
  per head: Q proj -> rope -> qT; scoresT = kT.kb^T-slice @ qT (PSUM f32);
  mask+scale (DVE) where needed; probsT = exp(. - 4) fp16 (ACT);
  AV: y[q, hd|sum] = sum_kb probsT_kb^T @ [V|1]; normalize by sum; transpose -> yT
  out[q, dm] = sum_h yT_h^T @ wo_h -> f32 out rows.

Causal structure is exploited only when the mask is exactly causal(-1e9);
otherwise a general variant computes all blocks (mask applied everywhere).
"""
import math

import numpy as np

B, S, D = 4, 2048, 2048
H, KH, HD = 16, 4, 128
DC = D // 128            # 16 contraction chunks
NKB = S // 128           # 16 key blocks
OWN = 1024               # owned q rows per core
NCORES = 8
SCALE = 1.0 / math.sqrt(HD)
EXP_BIAS = -4.0

_cache = {}


def _build(causal: bool):
    import concourse.bacc as bacc
    import concourse.tile as tile
    import concourse.mybir as mybir

    f16, f32 = mybir.dt.float16, mybir.dt.float32
    Alu = mybir.AluOpType
    Act = mybir.ActivationFunctionType

    if causal:
        KBSET = {0: [0, 1, 2, 3, 8, 9, 10, 11], 1: list(range(16))}
        STT = {0: set(KBSET[0]), 1: {4, 5, 6, 7, 12, 13, 14, 15}}
    else:
        KBSET = {0: list(range(16)), 1: list(range(16))}
        STT = {0: set(range(16)), 1: set(range(16))}

    nc = bacc.Bacc("TRN2", target_bir_lowering=False, debug=False,
                   num_devices=NCORES)

    xp = nc.dram_tensor("xp", [S, D], f16, kind="ExternalInput").ap()
    wq = nc.dram_tensor("wq", [D, H * HD], f16, kind="ExternalInput").ap()
    wk = nc.dram_tensor("wk", [D, KH * HD], f16, kind="ExternalInput").ap()
    wv = nc.dram_tensor("wv", [D, KH * HD], f16, kind="ExternalInput").ap()
    wo = nc.dram_tensor("wo", [H * HD, D], f16, kind="ExternalInput").ap()
    maskt = nc.dram_tensor("maskt", [S, OWN], f32, kind="ExternalInput").ap()
    c2 = nc.dram_tensor("c2", [128, S], f16, kind="ExternalInput").ap()
    s2 = nc.dram_tensor("s2", [128, S], f16, kind="ExternalInput").ap()
    swp = nc.dram_tensor("swp", [128, 128], f16, kind="ExternalInput").ap()
    ident = nc.dram_tensor("ident", [128, 128], f16, kind="ExternalInput").ap()
    outp = nc.dram_tensor("outp", [OWN, D], f32, kind="ExternalOutput").ap()

    yT_d = nc.dram_tensor("yT_d", [H, 128, OWN], f16)  # internal staging

    with tile.TileContext(nc) as tc:
        with tc.tile_pool(name="const", bufs=1) as constp, \
             tc.tile_pool(name="resid", bufs=1) as resid:
            swpt = constp.tile([128, 128], f16)
            identt = constp.tile([128, 128], f16)
            bias_t = constp.tile([128, 1], f32)
            c2t = constp.tile([128, S], f16)
            s2t = constp.tile([128, S], f16)
            nc.sync.dma_start(out=swpt, in_=swp)
            nc.sync.dma_start(out=identt, in_=ident)
            nc.sync.dma_start(out=c2t, in_=c2)
            nc.sync.dma_start(out=s2t, in_=s2)
            nc.vector.memset(bias_t, EXP_BIAS)

            kT = resid.tile([128, KH, S], f16)            # [hd, kv, s]
            V = resid.tile([128, NKB, KH, HD + 1], f16)   # [s128, kb, kv, hd|1]
            for kb in range(NKB):
                nc.vector.memset(V[:, kb, :, HD:HD + 1], 1.0)

            with tc.tile_pool(name="ph12", bufs=1) as ph, \
                 tc.tile_pool(name="psA", bufs=1, space="PSUM") as psA:
                xTq = ph.tile([128, DC, OWN], f16)   # owned-q cols of xT, resident
                wkt = ph.tile([128, DC, KH * HD], f16)
                wvt = ph.tile([128, DC, KH * HD], f16)
                nc.sync.dma_start(out=wkt, in_=wk.rearrange("(c p) n -> p c n", p=128))
                nc.sync.dma_start(out=wvt, in_=wv.rearrange("(c p) n -> p c n", p=128))

                def rope_evict(pP, out_ap, n, tag):
                    """rope: out = pP*c2 + (SWP@fp16(pP))*s2 over [128, n] cols at
                    column-offset `off` of the rope tables."""
                    off, ncols = n
                    psb = ph.tile([128, 512], f16, name=f"psb_{tag}", tag="psb", bufs=2)
                    nc.scalar.copy(out=psb[:, 0:ncols], in_=pP)
                    pSw = psA.tile([128, 512], f32, name=f"pSw_{tag}", tag="pSw", bufs=1)
                    nc.tensor.matmul(pSw[:, 0:ncols], swpt, psb[:, 0:ncols],
                                     start=True, stop=True)
                    m1 = ph.tile([128, 512], f32, name=f"m1_{tag}", tag="m1", bufs=2)
                    m2 = ph.tile([128, 512], f32, name=f"m2_{tag}", tag="m2", bufs=2)
                    nc.vector.tensor_mul(m1[:, 0:ncols], pP, c2t[:, off:off + ncols])
                    nc.vector.tensor_mul(m2[:, 0:ncols], pSw[:, 0:ncols],
                                         s2t[:, off:off + ncols])
                    nc.vector.tensor_add(out_ap, m1[:, 0:ncols], m2[:, 0:ncols])

                # ---- Phase 1: xT via DMA-transpose; K/V projections ----
                for sc in range(4):
                    if sc < 2:
                        xs = [xTq[:, dc, sc * 512:(sc + 1) * 512] for dc in range(DC)]
                    else:
                        xr = ph.tile([128, DC, 512], f16, name=f"xr{sc}", tag="xr",
                                     bufs=2)
                        xs = [xr[:, dc, :] for dc in range(DC)]
                    for dc in range(DC):
                        nc.sync.dma_start_transpose(
                            xs[dc], xp[sc * 512:(sc + 1) * 512, dc * 128:(dc + 1) * 128])
                    # K projection + rope -> kT
                    for kv in range(KH):
                        kP = psA.tile([128, 512], f32, name=f"kP{sc}_{kv}", tag="pp",
                                      bufs=2)
                        for dc in range(DC):
                            nc.tensor.matmul(kP, wkt[:, dc, kv * HD:(kv + 1) * HD],
                                             xs[dc], start=(dc == 0), stop=(dc == DC - 1))
                        rope_evict(kP, kT[:, kv, sc * 512:(sc + 1) * 512],
                                   (sc * 512, 512), f"k{sc}_{kv}")
                    # V projection (natural layout): lhsT = xT slice, rhs = wv
                    for sb in range(4):
                        kb = sc * 4 + sb
                        vP = psA.tile([128, 512], f32, name=f"vP{kb}", tag="pp", bufs=2)
                        for dc in range(DC):
                            nc.tensor.matmul(vP, xs[dc][:, sb * 128:(sb + 1) * 128],
                                             wvt[:, dc, :], start=(dc == 0),
                                             stop=(dc == DC - 1))
                        nc.scalar.copy(
                            out=V[:, kb, :, 0:HD],
                            in_=vP.rearrange("p (kv h) -> p kv h", kv=KH))

                # ---- Phase 2+3: per head Q proj + rope + attention ----
                for h in range(H):
                    kv = h % KH
                    wqt = ph.tile([128, DC, HD], f16, name=f"wq{h}", tag="wq", bufs=2)
                    nc.sync.dma_start(
                        out=wqt,
                        in_=wq[:, h * HD:(h + 1) * HD].rearrange("(c p) n -> p c n", p=128))
                    qT = ph.tile([128, OWN], f16, name=f"qT{h}", tag="qT", bufs=2)
                    for qc in range(2):
                        qP = psA.tile([128, 512], f32, name=f"qP{h}_{qc}", tag="pp",
                                      bufs=2)
                        for dc in range(DC):
                            nc.tensor.matmul(qP, wqt[:, dc, :],
                                             xTq[:, dc, qc * 512:(qc + 1) * 512],
                                             start=(dc == 0), stop=(dc == DC - 1))
                        rope_evict(qP, qT[:, qc * 512:(qc + 1) * 512],
                                   (qc * 512, 512), f"q{h}_{qc}")
                    for ch in range(2):
                        kbs = KBSET[ch]
                        probs = ph.tile([128, 16, 512], f16, name=f"pr{h}_{ch}",
                                        tag="probs", bufs=1)
                        for j, kb in enumerate(kbs):
                            sc_ps = psA.tile([128, 512], f32, name=f"sc{h}_{ch}_{kb}",
                                             tag="sc", bufs=2)
                            nc.tensor.matmul(sc_ps, kT[:, kv, kb * 128:(kb + 1) * 128],
                                             qT[:, ch * 512:(ch + 1) * 512],
                                             start=True, stop=True)
                            if kb in STT[ch]:
                                mt = ph.tile([128, 512], f32, name=f"mt{h}_{ch}_{kb}",
                                             tag="mt", bufs=3)
                                nc.sync.dma_start(
                                    out=mt, in_=maskt[kb * 128:(kb + 1) * 128,
                                                      ch * 512:(ch + 1) * 512])
                                scm = ph.tile([128, 512], f32, name=f"scm{h}_{ch}_{kb}",
                                              tag="scm", bufs=2)
                                nc.vector.scalar_tensor_tensor(
                                    out=scm, in0=sc_ps, scalar=SCALE, in1=mt,
                                    op0=Alu.mult, op1=Alu.add)
                                nc.scalar.activation(out=probs[:, j, :], in_=scm,
                                                     func=Act.Exp, bias=bias_t, scale=1.0)
                            else:
                                nc.scalar.activation(out=probs[:, j, :], in_=sc_ps,
                                                     func=Act.Exp, bias=bias_t,
                                                     scale=SCALE)
                        for qs in range(4):
                            yP = psA.tile([128, HD + 1], f32, name=f"yP{h}_{ch}_{qs}",
                                          tag="yP", bufs=2)
                            for j, kb in enumerate(kbs):
                                nc.tensor.matmul(yP, probs[:, j, qs * 128:(qs + 1) * 128],
                                                 V[:, kb, kv, :], start=(j == 0),
                                                 stop=(j == len(kbs) - 1))
                            rc = ph.tile([128, 1], f32, name=f"rc{h}_{ch}_{qs}",
                                         tag="rc", bufs=2)
                            nc.vector.reciprocal(rc, yP[:, HD:HD + 1])
                            ysb = ph.tile([128, HD], f16, name=f"ysb{h}_{ch}_{qs}",
                                          tag="ysb", bufs=2)
                            nc.vector.tensor_scalar_mul(ysb, yP[:, 0:HD], rc)
                            yTp = psA.tile([128, 128], f16, name=f"yTp{h}_{ch}_{qs}",
                                           tag="yTp", bufs=1)
                            nc.tensor.transpose(yTp, ysb, identt)
                            yTs = ph.tile([128, 128], f16, name=f"yTs{h}_{ch}_{qs}",
                                          tag="yTs", bufs=2)
                            nc.vector.tensor_copy(out=yTs, in_=yTp)
                            nc.sync.dma_start(
                                out=yT_d[h, :, ch * 512 + qs * 128:ch * 512 + (qs + 1) * 128],
                                in_=yTs)

            # ---- Phase 4: output projection ----
            with tc.tile_pool(name="p4", bufs=1) as p4, \
                 tc.tile_pool(name="psB", bufs=1, space="PSUM") as psB:
                wot = p4.tile([128, DC, 4, 512], f16)  # [hd128, h, dmc, dm]
                nc.sync.dma_start(
                    out=wot,
                    in_=wo.rearrange("(c p) (m n) -> p c m n", p=128, n=512))
                for qs in range(8):
                    yTq = p4.tile([128, H, 128], f16, name=f"yTq{qs}", tag="yTq", bufs=2)
                    nc.sync.dma_start(
                        out=yTq, in_=yT_d.rearrange("h p n -> p h n")[:, :,
                                                    qs * 128:(qs + 1) * 128])
                    for dmc in range(4):
                        oP = psB.tile([128, 512], f32, name=f"oP{qs}_{dmc}", tag="oP",
                                      bufs=2)
                        for h in range(H):
                            nc.tensor.matmul(oP, yTq[:, h, :], wot[:, h, dmc, :],
                                             start=(h == 0), stop=(h == H - 1))
                        osb = p4.tile([128, 512], f32, name=f"osb{qs}_{dmc}", tag="osb",
                                      bufs=2)
                        nc.vector.tensor_copy(out=osb, in_=oP)
                        nc.sync.dma_start(
                            out=outp[qs * 128:(qs + 1) * 128, dmc * 512:(dmc + 1) * 512],
                            in_=osb)

    nc.compile()
    return nc


def _perm_rows(parity: int) -> np.ndarray:
    chunks = [0, 3, 1, 2] if parity == 0 else [1, 2, 0, 3]
    return np.concatenate([np.arange(c * 512, (c + 1) * 512) for c in chunks])


def _host_prep(x, wq, wk, wv, wo, freqs_cos, freqs_sin, mask, causal):
    f16 = np.float16
    swp_np = np.zeros((128, 128), dtype=f16)
    idx = np.arange(64)
    swp_np[2 * idx, 2 * idx + 1] = 1.0
    swp_np[2 * idx + 1, 2 * idx] = 1.0
    id_np = np.eye(128, dtype=f16)
    sign = np.tile(np.array([-1.0, 1.0], np.float32), 64)[:, None]

    shared = {
        "wq": np.ascontiguousarray(wq.astype(f16)),
        "wk": np.ascontiguousarray(wk.astype(f16)),
        "wv": np.ascontiguousarray(wv.astype(f16)),
        "wo": np.ascontiguousarray(wo.astype(f16)),
        "swp": swp_np, "ident": id_np,
    }
    in_maps = []
    perms = []
    for core in range(NCORES):
        b, parity = core // 2, core % 2
        perm = _perm_rows(parity)
        owned = perm[:OWN]
        cosP = np.repeat(freqs_cos[perm].T, 2, axis=0)          # [128, S]
        sinP = np.repeat(freqs_sin[perm].T, 2, axis=0) * sign   # [128, S]
        m = {
            "xp": np.ascontiguousarray(x[b][perm].astype(f16)),
            "maskt": np.ascontiguousarray(mask[np.ix_(owned, perm)].T.astype(np.float32)),
            "c2": np.ascontiguousarray(cosP.astype(f16)),
            "s2": np.ascontiguousarray(sinP.astype(f16)),
            **shared,
        }
        in_maps.append(m)
        perms.append(owned)
    return in_maps, perms


def _is_causal(mask: np.ndarray) -> bool:
    if mask.shape != (S, S):
        return False
    iu = np.triu_indices(S, k=1)
    if not np.all(mask[iu] <= -1e8):
        return False
    il = np.tril_indices(S, k=0)
    return bool(np.all(mask[il] == 0.0))


def run(x, wq, wk, wv, wo, freqs_cos, freqs_sin, mask, trace=False):
    from concourse.bass_utils import run_bass_kernel_spmd

    causal = _is_causal(np.asarray(mask))
    key = ("causal" if causal else "general")
    if key not in _cache:
        _cache[key] = _build(causal)
    nc = _cache[key]

    in_maps, owneds = _host_prep(
        np.asarray(x, np.float32), np.asarray(wq, np.float32),
        np.asarray(wk, np.float32), np.asarray(wv, np.float32),
        np.asarray(wo, np.float32), np.asarray(freqs_cos, np.float32),
        np.asarray(freqs_sin, np.float32), np.asarray(mask, np.float32), causal)

    res = run_bass_kernel_spmd(nc, in_maps, list(range(NCORES)), trace=trace)

    out = np.empty((B, S, D), dtype=np.float32)
    for core in range(NCORES):
        b = core // 2
        out[b, owneds[core], :] = res.results[core]["outp"]
    return out, res


def kernel(x, wq, wk, wv, wo, freqs_cos, freqs_sin, mask):
    out, _ = run(x, wq, wk, wv, wo, freqs_cos, freqs_sin, mask, trace=False)
    return out


# revision 16
# speedup vs baseline: 1.3588x; 1.3588x over previous
"""GQA attention (B=4,S=2048,D=2048,H=16,KH=4) + RoPE + causal mask on 8 trn2 cores.

Sharding: 8 cores = 4 batches x 2 head-groups (8 heads each). Every core
computes K/V for the full sequence (kv-head mapping h%4 is identical for both
groups), attention for its 8 heads over all 2048 q rows with block-causal
skipping, and a partial output projection over its heads; the host sums the
two partial outputs per batch.

Per-core pipeline (fp16 matmuls, fp32 accumulate/softmax):
  x fp16 --DMA-transpose--> xT [d, s] resident
  K/V projections -> rope(K) -> kT [hd, s] fp16; V [s128, kb, kv, hd|1] fp16
  per head: Q proj -> rope -> qT [hd, 2048]
  per q-chunk qc (512 wide): kbs = 0..4qc+3 (causal) or all (general)
    scoresT[kb] = kT-slice^T @ qT-chunk (PSUM f32)
    diagonal band: (scores*scale + mask) on DVE; else exp straight from PSUM
    probsT = exp(.-4) fp16 (ACT, paired tiles)
    AV: y[q, hd|sum] = sum_kb probsT_kb-slice^T @ [V|1]; normalize; PE-transpose
  out_partial[q, dm] = sum_{local h} yT_h^T @ wo_h -> f32 (host adds pairs)

Causal block-skipping only when the mask is exactly causal(-1e9); otherwise a
general variant computes and masks every block.
"""
import math

import numpy as np

B, S, D = 4, 2048, 2048
H, KH, HD = 16, 4, 128
HL = 8                   # heads per core
DC = D // 128            # contraction chunks
NKB = S // 128           # key blocks
NQC = S // 512           # q chunks
NCORES = 8
SCALE = 1.0 / math.sqrt(HD)
EXP_BIAS = -4.0

_cache = {}


def _build(causal: bool):
    import concourse.bacc as bacc
    import concourse.tile as tile
    import concourse.mybir as mybir

    f16, f32 = mybir.dt.float16, mybir.dt.float32
    Alu = mybir.AluOpType
    Act = mybir.ActivationFunctionType

    nc = bacc.Bacc("TRN2", target_bir_lowering=False, debug=False,
                   num_devices=NCORES)

    xt = nc.dram_tensor("xt", [D, S], f16, kind="ExternalInput").ap()
    wqg = nc.dram_tensor("wqg", [D, HL * HD], f16, kind="ExternalInput").ap()
    wk = nc.dram_tensor("wk", [D, KH * HD], f16, kind="ExternalInput").ap()
    wv = nc.dram_tensor("wv", [D, KH * HD], f16, kind="ExternalInput").ap()
    wog = nc.dram_tensor("wog", [HL * HD, D], f16, kind="ExternalInput").ap()
    # fp16 mask in pre-scale score units (clamped to +-1e4; exp underflow
    # to exactly 0 matches the reference's exp(-1e9)).
    # causal: the 4 diagonal-band patterns [p, i, q']; general: [p, kb, q]
    mshape = [128, 4, 512] if causal else [128, NKB, S]
    maskt = nc.dram_tensor("maskt", mshape, f16, kind="ExternalInput").ap()
    c2 = nc.dram_tensor("c2", [128, S], f16, kind="ExternalInput").ap()
    s2 = nc.dram_tensor("s2", [128, S], f16, kind="ExternalInput").ap()
    swp = nc.dram_tensor("swp", [128, 128], f16, kind="ExternalInput").ap()
    ident = nc.dram_tensor("ident", [128, 128], f16, kind="ExternalInput").ap()
    outp = nc.dram_tensor("outp", [S, D], f32, kind="ExternalOutput").ap()


    with tile.TileContext(nc) as tc:
        with tc.tile_pool(name="const", bufs=1) as constp, \
             tc.tile_pool(name="resid", bufs=1) as resid, \
             tc.tile_pool(name="psA", bufs=1, space="PSUM") as psA:
            swpt = constp.tile([128, 128], f16)
            identt = constp.tile([128, 128], f16)
            bias_t = constp.tile([128, 1], f32)
            nc.sync.dma_start(out=swpt, in_=swp)
            nc.sync.dma_start(out=identt, in_=ident)
            nc.vector.memset(bias_t, EXP_BIAS)

            kT = resid.tile([128, KH, S], f16)            # [hd, kv, s]
            V = resid.tile([128, NKB, KH, HD + 1], f16)   # [s128, kb, kv, hd|1]
            qTs = resid.tile([128, HL, S], f16)           # [hd, h, s]
            for kb in range(NKB):
                nc.vector.memset(V[:, kb, :, HD:HD + 1], 1.0)

            with tc.tile_pool(name="p_x", bufs=1) as p_x:
                xT = p_x.tile([128, DC, S], f16)          # [d128, dc, s]
                c2t = p_x.tile([128, S], f16)
                s2t = p_x.tile([128, S], f16)
                nc.sync.dma_start(out=c2t, in_=c2)
                nc.sync.dma_start(out=s2t, in_=s2)

                def rope_evict(pP, out_ap, off, ncols, tag):
                    """out = pP*c2 + (SWP @ fp16(pP))*s2, table cols [off, off+ncols)."""
                    psb = p_x.tile([128, 512], f16, name=f"psb_{tag}", tag="psb",
                                   bufs=2)
                    nc.scalar.copy(out=psb[:, 0:ncols], in_=pP)
                    pSw = psA.tile([128, 512], f32, name=f"pSw_{tag}", tag="aux",
                                   bufs=2)
                    nc.tensor.matmul(pSw[:, 0:ncols], swpt, psb[:, 0:ncols],
                                     start=True, stop=True)
                    m1 = p_x.tile([128, 512], f32, name=f"m1_{tag}", tag="m1", bufs=2)
                    m2 = p_x.tile([128, 512], f32, name=f"m2_{tag}", tag="m2", bufs=2)
                    nc.vector.tensor_mul(m1[:, 0:ncols], pP, c2t[:, off:off + ncols])
                    nc.vector.tensor_mul(m2[:, 0:ncols], pSw[:, 0:ncols],
                                         s2t[:, off:off + ncols])
                    nc.gpsimd.tensor_add(out_ap, m1[:, 0:ncols], m2[:, 0:ncols])

                # ---- Phase 1: K/V projections ----
                with tc.tile_pool(name="p_kv", bufs=1) as p_kv:
                    wkt = p_kv.tile([128, DC, KH * HD], f16)
                    wvt = p_kv.tile([128, DC, KH * HD], f16)
                    # x arrives pre-transposed from the host: plain copies
                    # only (DMATranspose would serialize the DMA subsystem
                    # against every concurrent copy).
                    for dc in range(DC):
                        nc.sync.dma_start(out=wkt[:, dc, :],
                                          in_=wk[dc * 128:(dc + 1) * 128, :])
                        nc.sync.dma_start(out=wvt[:, dc, :],
                                          in_=wv[dc * 128:(dc + 1) * 128, :])
                        nc.sync.dma_start(out=xT[:, dc, :],
                                          in_=xt[dc * 128:(dc + 1) * 128, :])
                    for sc in range(4):
                        cs = slice(sc * 512, (sc + 1) * 512)
                        for kv in range(KH):
                            kP = psA.tile([128, 512], f32, name=f"kP{sc}_{kv}",
                                          tag="big", bufs=4)
                            for dc in range(DC):
                                nc.tensor.matmul(kP, wkt[:, dc, kv * HD:(kv + 1) * HD],
                                                 xT[:, dc, cs], start=(dc == 0),
                                                 stop=(dc == DC - 1))
                            rope_evict(kP, kT[:, kv, cs], sc * 512, 512, f"k{sc}_{kv}")
                        for sb in range(4):
                            kb = sc * 4 + sb
                            vP = psA.tile([128, 512], f32, name=f"vP{kb}", tag="big",
                                          bufs=4)
                            for dc in range(DC):
                                nc.tensor.matmul(
                                    vP,
                                    xT[:, dc, sc * 512 + sb * 128:sc * 512 + (sb + 1) * 128],
                                    wvt[:, dc, :], start=(dc == 0),
                                    stop=(dc == DC - 1))
                            nc.scalar.copy(
                                out=V[:, kb, :, 0:HD],
                                in_=vP.rearrange("p (kv h) -> p kv h", kv=KH))

                # ---- Phase 2: all Q projections + rope ----
                for h in range(HL):
                    wqt = p_x.tile([128, DC, HD], f16, name=f"wq{h}", tag="wq", bufs=2)
                    nc.sync.dma_start(
                        out=wqt,
                        in_=wqg[:, h * HD:(h + 1) * HD].rearrange("(c p) n -> p c n",
                                                                  p=128))
                    for qc in range(NQC):
                        qP = psA.tile([128, 512], f32, name=f"qP{h}_{qc}", tag="big",
                                      bufs=4)
                        for dc in range(DC):
                            nc.tensor.matmul(qP, wqt[:, dc, :],
                                             xT[:, dc, qc * 512:(qc + 1) * 512],
                                             start=(dc == 0), stop=(dc == DC - 1))
                        rope_evict(qP, qTs[:, h, qc * 512:(qc + 1) * 512],
                                   qc * 512, 512, f"q{h}_{qc}")

            # ---- Phase 3: attention; Phase 4: output projection ----
            with tc.tile_pool(name="p_att", bufs=1) as ph, \
                 tc.tile_pool(name="p_4", bufs=1) as p4:
                mconst = None
                if causal:
                    mconst = ph.tile([128, 4, 512], f16)
                    nc.sync.dma_start(out=mconst, in_=maskt)
                wot = p4.tile([128, DC // 2, 4, 512], f16)  # [hd128, h, dmc, dm]
                nc.sync.dma_start(
                    out=wot,
                    in_=wog.rearrange("(c p) (m n) -> p c m n", p=128, n=512))

                def out_proj(qc, yTsb):
                    # output projection for one q-chunk (all local heads)
                    for qsl in range(4):
                        qs = qc * 4 + qsl
                        for dmc in range(4):
                            oP = psA.tile([128, 512], f32, name=f"oP{qs}_{dmc}",
                                          tag="big", bufs=4)
                            for h in range(HL):
                                nc.tensor.matmul(
                                    oP, yTsb[:, h, qsl * 128:(qsl + 1) * 128],
                                    wot[:, h, dmc, :],
                                    start=(h == 0), stop=(h == HL - 1))
                            osb = p4.tile([128, 512], f32, name=f"osb{qs}_{dmc}",
                                          tag="osb", bufs=2)
                            nc.vector.tensor_copy(out=osb, in_=oP)
                            nc.sync.dma_start(
                                out=outp[qs * 128:(qs + 1) * 128,
                                         dmc * 512:(dmc + 1) * 512],
                                in_=osb)

                pending = None
                for qc in range(NQC):
                    yTsb = p4.tile([128, HL, 512], f16, name=f"yTsb{qc}",
                                   tag="yTsb", bufs=2)
                    mqc = None
                    if not causal:
                        mqc = ph.tile([128, NKB, 512], f16, name=f"mqc{qc}",
                                      tag="mqc", bufs=2)
                        nc.sync.dma_start(out=mqc,
                                          in_=maskt[:, :, qc * 512:(qc + 1) * 512])
                    for h in range(HL):
                        kv = h % KH
                        kbs = list(range(4 * qc + 4)) if causal else list(range(NKB))
                        diag = set(range(4 * qc, 4 * qc + 4)) if causal \
                            else set(range(NKB))
                        probs = ph.tile([128, 16, 512], f16, name=f"pr{h}_{qc}",
                                        tag="probs", bufs=2)
                        for j, kb in enumerate(kbs):
                            sc_ps = psA.tile([128, 512], f32, name=f"sc{h}_{qc}_{kb}",
                                             tag="big", bufs=4)
                            masked = kb in diag
                            nc.tensor.matmul(sc_ps, kT[:, kv, kb * 128:(kb + 1) * 128],
                                             qTs[:, h, qc * 512:(qc + 1) * 512],
                                             start=True, stop=not masked)
                            if masked:
                                # accumulate the additive mask on the PE
                                if causal:
                                    mrhs = mconst[:, kb - 4 * qc, :]
                                else:
                                    mrhs = mqc[:, kb, :]
                                nc.tensor.matmul(sc_ps, identt, mrhs,
                                                 start=False, stop=True)
                            nc.scalar.activation(out=probs[:, j, :], in_=sc_ps,
                                                 func=Act.Exp, bias=bias_t,
                                                 scale=SCALE)
                        ysbs = []
                        for qs in range(4):
                            yP = psA.tile([128, HD + 1], f32, name=f"yP{h}_{qc}_{qs}",
                                          tag="yP", bufs=2)
                            for j, kb in enumerate(kbs):
                                nc.tensor.matmul(yP,
                                                 probs[:, j, qs * 128:(qs + 1) * 128],
                                                 V[:, kb, kv, :], start=(j == 0),
                                                 stop=(j == len(kbs) - 1))
                            rc = ph.tile([128, 1], f32, name=f"rc{h}_{qc}_{qs}",
                                         tag="rc", bufs=2)
                            nc.vector.reciprocal(rc, yP[:, HD:HD + 1])
                            ysb = ph.tile([128, HD], f16, name=f"ysb{h}_{qc}_{qs}",
                                          tag="ysb", bufs=5)
                            nc.vector.tensor_scalar_mul(ysb, yP[:, 0:HD], rc)
                            ysbs.append(ysb)
                        for qs in range(4):
                            yTp = psA.tile([128, 512], f16, name=f"yTp{h}_{qc}_{qs}",
                                           tag="aux", bufs=2)
                            nc.tensor.transpose(yTp[:, 0:128], ysbs[qs], identt)
                            nc.vector.tensor_copy(
                                out=yTsb[:, h, qs * 128:(qs + 1) * 128],
                                in_=yTp[:, 0:128])
                        if h == 0 and pending is not None:
                            out_proj(*pending)
                            pending = None

                    pending = (qc, yTsb)
                if pending is not None:
                    out_proj(*pending)

    nc.compile()
    return nc


def _host_prep(x, wq, wk, wv, wo, freqs_cos, freqs_sin, mask, causal):
    f16 = np.float16
    swp_np = np.zeros((128, 128), dtype=f16)
    idx = np.arange(64)
    swp_np[2 * idx, 2 * idx + 1] = 1.0
    swp_np[2 * idx + 1, 2 * idx] = 1.0
    id_np = np.eye(128, dtype=f16)
    sign = np.tile(np.array([-1.0, 1.0], np.float32), 64)[:, None]
    c2_np = np.ascontiguousarray(np.repeat(freqs_cos.T, 2, axis=0).astype(f16))
    s2_np = np.ascontiguousarray(
        (np.repeat(freqs_sin.T, 2, axis=0) * sign).astype(f16))

    if causal:
        # 4 canonical diagonal-band patterns in pre-scale units: -1e4 gives
        # exp((s-1e4)*scale-4) == 0 exactly in f32, matching exp(-1e9)
        p = np.arange(128)[:, None, None]
        i = np.arange(4)[None, :, None]
        qq = np.arange(512)[None, None, :]
        mt = np.where(i * 128 + p > qq, -1e4, 0.0).astype(f16)
    else:
        mt = np.clip(mask.astype(np.float64) / SCALE, -1e4, 1e4).astype(f16)
        mt = mt.reshape(NKB, 128, S).transpose(1, 0, 2)
    mt = np.ascontiguousarray(mt)

    shared = {
        "wk": np.ascontiguousarray(wk.astype(f16)),
        "wv": np.ascontiguousarray(wv.astype(f16)),
        "maskt": mt, "c2": c2_np, "s2": s2_np,
        "swp": swp_np, "ident": id_np,
    }
    xb = [np.ascontiguousarray(x[b].astype(f16).T) for b in range(B)]
    wqg = [np.ascontiguousarray(wq[:, g * HL * HD:(g + 1) * HL * HD].astype(f16))
           for g in range(2)]
    wog = [np.ascontiguousarray(wo[g * HL * HD:(g + 1) * HL * HD, :].astype(f16))
           for g in range(2)]
    in_maps = []
    for core in range(NCORES):
        b, g = core // 2, core % 2
        in_maps.append({"xt": xb[b], "wqg": wqg[g], "wog": wog[g], **shared})
    return in_maps


def _is_causal(mask: np.ndarray) -> bool:
    if mask.shape != (S, S):
        return False
    iu = np.triu_indices(S, k=1)
    if not np.all(mask[iu] <= -1e8):
        return False
    il = np.tril_indices(S, k=0)
    return bool(np.all(mask[il] == 0.0))


def run(x, wq, wk, wv, wo, freqs_cos, freqs_sin, mask, trace=False):
    from concourse.bass_utils import run_bass_kernel_spmd

    causal = _is_causal(np.asarray(mask))
    key = "causal" if causal else "general"
    if key not in _cache:
        _cache[key] = _build(causal)
    nc = _cache[key]

    in_maps = _host_prep(
        np.asarray(x, np.float32), np.asarray(wq, np.float32),
        np.asarray(wk, np.float32), np.asarray(wv, np.float32),
        np.asarray(wo, np.float32), np.asarray(freqs_cos, np.float32),
        np.asarray(freqs_sin, np.float32), np.asarray(mask, np.float32), causal)

    res = run_bass_kernel_spmd(nc, in_maps, list(range(NCORES)), trace=trace)

    out = np.empty((B, S, D), dtype=np.float32)
    for b in range(B):
        out[b] = res.results[2 * b]["outp"] + res.results[2 * b + 1]["outp"]
    return out, res


def kernel(x, wq, wk, wv, wo, freqs_cos, freqs_sin, mask):
    out, _ = run(x, wq, wk, wv, wo, freqs_cos, freqs_sin, mask, trace=False)
    return out
